# revision 22
# baseline (speedup 1.0000x reference)
"""Trainium2 Bass kernel for nn_EdgeConvolution (gnn_message_passing).

Math
----
Reference (B=2, N=512, C=128, U=128), adj binary {0,1}:
  masked[b,i,j,:]  = adj[b,i,j] * x[b,i,:]
  a_sel[b,i]       = adj[b,i, xidx[b,i]]
  edging[b,i,j,:]  = adj[b,i,j] * [ x_i | (a_sel_i - 1)*x_i ]      (adj^2 = adj)
  out[b,i,j,:]     = relu(adj*(u_i + (a_sel_i-1)*v_i) + b), u = x@W1, v = x@W2
Over j there are only two values per (b,i):
  z1_i = relu(u_i + (a_sel_i-1)*v_i + b)   (edges with adj=1, count k_i)
  z0   = relu(b)                            (edges with adj=0, count N-k_i)
  maxp_i    = max(h1_i*z1_i, h0_i*z0),  h1 = [k>0], h0 = [k<N]
  n_i       = k_i*s1_i + (N-k_i)*s0,  s1 = [any z1>0], s0 = [any z0>0]
  avgpool_i = [ k_i*x_i | k_i*(a_sel_i-1)*x_i ] / n_i
Per-core slab: 128 of the 1024 (b,i) rows; w/b replicated.

Implementation notes
--------------------
- Two packed input DMAs per core: pakA = adj (fp8 e4m3, exact for 0/1) on
  the SP HWDGE ring; pakB = [x^T | Wcat | x | b-broadcast | xidx] in fp16
  on the ACT ring.
- +b is folded into the matmul as a rank-1 accumulate (ones x b_row).
- h1 is folded into the relu (z1 = max(zz*h1, 0)); h0 scales z0 on ACT.
- Single full-row output store issued from ACT; no engine waits on the
  store completion — its receipt latency hides under the runtime's fixed
  end-of-NEFF semaphore-zeroing postamble.
- No same-engine self-waits: engine execution is serial in-order (accum
  reads materialize via in-stream READ_ACCUMULATOR), so only cross-engine
  and DMA waits are needed.
"""

import numpy as np

B, N, C, U = 2, 512, 128, 128
P = 128          # rows (b,i) per core == SBUF partitions
NCORES = 8
OUTF = U + 2 * C  # 384
PAKA_W = 528     # adj 512 | xidx 1 | pad 15   (fp16)
PAKB_W = 640     # xT 128 | wcat 256 | x 128 | b_bcast 128

_CACHE: dict = {}


def _build_nc():
    import concourse.bacc as bacc
    import concourse.mybir as mybir

    f32 = mybir.dt.float32
    f16 = mybir.dt.float16
    f8 = mybir.dt.float8e4
    Alu = mybir.AluOpType
    Act = mybir.ActivationFunctionType

    nc = bacc.Bacc("TRN2", target_bir_lowering=False, debug=False,
                   num_devices=NCORES)

    pakA_d = nc.dram_tensor("pakA", [P, PAKA_W], f16, kind="ExternalInput")
    pakB_d = nc.dram_tensor("pakB", [P, PAKB_W], f16, kind="ExternalInput")
    out_d = nc.dram_tensor("out", [P, OUTF], f16, kind="ExternalOutput")

    ctx_tensors = [
        ("pakA_t", [P, PAKA_W], f16), ("pakB_t", [P, PAKB_W], f16),
        ("iota16", [P, N], f16), ("ones16", [1, P], f16),
        ("scr", [P, N], f16), ("scr2", [P, N], f16), ("warm", [P, 1], f32),
        ("a_sel", [P, 1], f32), ("asm1", [P, 1], f32), ("k", [P, 1], f32),
        ("t_sb", [P, U], f32),
        ("zz", [P, U], f32), ("z1", [P, U], f16), ("z1sum", [P, 1], f32),
        ("z0", [P, U], f32), ("z0sum", [P, 1], f32), ("z0h", [P, U], f16),
        ("s0", [P, 1], f32), ("s1", [P, 1], f32), ("nk", [P, 1], f32),
        ("h0", [P, 1], f32), ("h1", [P, 1], f32), ("t2", [P, 1], f32),
        ("nn", [P, 1], f32), ("rn", [P, 1], f32),
        ("xcat", [P, 2 * C], f16),
        ("out_t", [P, OUTF], f16),
    ]

    from contextlib import ExitStack
    with ExitStack() as ctx:
        t = {}
        for name, shape, dt in ctx_tensors:
            t[name] = ctx.enter_context(nc.sbuf_tensor(name, shape, dt))
        mm = ctx.enter_context(nc.psum_tensor("mm", [P, 2 * U], f32))

        dA = ctx.enter_context(nc.semaphore("dA"))
        dB = ctx.enter_context(nc.semaphore("dB"))
        dS = ctx.enter_context(nc.semaphore("dS"))    # stores; never waited
        sV = ctx.enter_context(nc.semaphore("sV"))
        sPo = ctx.enter_context(nc.semaphore("sPo"))
        sAc = ctx.enter_context(nc.semaphore("sAc"))
        sPe = ctx.enter_context(nc.semaphore("sPe"))

        ap = lambda h: h.ap()
        adj = t["pakA_t"].ap()[:, 0:N]
        xT = t["pakB_t"].ap()[:, 0:C]
        wcat = t["pakB_t"].ap()[:, C:C + 2 * U]
        xrow = t["pakB_t"].ap()[:, C + 2 * U:C + 2 * U + C]
        bb = t["pakB_t"].ap()[:, C + 2 * U + C:C + 2 * U + 2 * C]
        brow = t["pakB_t"].ap()[0:1, C + 2 * U + C:C + 2 * U + 2 * C]
        xidx = t["pakA_t"].ap()[:, N:N + 1]
        c0 = nc.const_aps.aps[(f32, 0.0)]

        # ---- SP: input DMA A only --------------------------------------
        nc.sync.dma_start(ap(t["pakA_t"]), pakA_d.ap()).then_inc(dA, 16)

        # ---- ACT: input DMA B, k, z0 path, xcat scales, avg, store -----
        nc.scalar.dma_start(ap(t["pakB_t"]), pakB_d.ap()).then_inc(dB, 16)
        # warm the activation table during the DMA wait
        nc.scalar.activation(out=ap(t["warm"]), in_=c0, func=Act.Relu,
                             bias=c0[:, 0:1])
        nc.scalar.wait_ge(dA, 16)
        nc.scalar.activation(out=ap(t["scr2"]), in_=adj, func=Act.Copy,
                             accum_out=t["k"].ap()[:, 0:1]
                             ).then_inc(sAc, 1)                       # ->1 k
        nc.scalar.wait_ge(dB, 16)
        nc.scalar.activation(out=ap(t["z0"]), in_=bb, func=Act.Relu,
                             bias=c0[:, 0:1],
                             accum_out=t["z0sum"].ap()[:, 0:1]
                             ).then_inc(sAc, 1)                       # ->2 z0
        nc.scalar.wait_ge(sAc, 1)            # k accum lands async
        nc.scalar.activation(out=t["xcat"].ap()[:, 0:C], in_=xrow,
                             func=Act.Copy, scale=t["k"].ap()[:, 0:1]
                             ).then_inc(sAc, 1)                       # ->3 xk
        nc.scalar.wait_ge(sAc, 3)            # xk visible (self)
        nc.scalar.wait_ge(sV, 2)             # asm1
        nc.scalar.activation(out=t["xcat"].ap()[:, C:2 * C],
                             in_=t["xcat"].ap()[:, 0:C],
                             func=Act.Copy, scale=t["asm1"].ap()[:, 0:1]
                             ).then_inc(sAc, 1)                       # ->4 xcat2
        nc.scalar.wait_ge(sAc, 4)            # xcat2 visible (self)
        nc.scalar.wait_ge(sV, 8)             # rn
        nc.scalar.activation(out=t["out_t"].ap()[:, U:OUTF],
                             in_=ap(t["xcat"]),
                             func=Act.Copy, scale=t["rn"].ap()[:, 0:1]
                             ).then_inc(sAc, 1)                       # ->5 avg
        nc.scalar.wait_ge(sAc, 5)            # avg visible (self)
        nc.scalar.wait_ge(sV, 10)            # out_max written by DVE
        nc.scalar.dma_start(out_d.ap(), ap(t["out_t"])).then_inc(dS, 16)

        # ---- PE: mm = [x@W1 + b | x@W2], v half first -------------------
        nc.tensor.wait_ge(dB, 16)
        nc.tensor.matmul(mm.ap()[:, U:2 * U], lhsT=xT,
                         rhs=wcat[:, U:2 * U],
                         start=True, stop=True).then_inc(sPe, 1)      # ->1 v
        nc.tensor.matmul(mm.ap()[:, 0:U], lhsT=xT, rhs=wcat[:, 0:U],
                         start=True, stop=False)
        nc.tensor.wait_ge(sPo, 1)            # ones16
        nc.tensor.matmul(mm.ap()[:, 0:U], lhsT=ap(t["ones16"]), rhs=brow,
                         start=False, stop=True,
                         skip_group_check=True).then_inc(sPe, 1)      # ->2 u+b

        # ---- GPSIMD: iota + k-derived scalars --------------------------
        nc.gpsimd.memset(ap(t["ones16"]), 1.0).then_inc(sPo, 1)       # ->1
        nc.gpsimd.iota(ap(t["iota16"]), pattern=[[1, N]], base=0,
                       channel_multiplier=0,
                       allow_small_or_imprecise_dtypes=True
                       ).then_inc(sPo, 1)                             # ->2
        nc.gpsimd.wait_ge(sAc, 1)            # k
        nc.gpsimd.tensor_scalar(out=ap(t["nk"]), in0=ap(t["k"]),
                                scalar1=-1.0, scalar2=float(N),
                                op0=Alu.mult, op1=Alu.add
                                ).then_inc(sPo, 1)                    # ->3
        nc.gpsimd.tensor_scalar(out=ap(t["h0"]), in0=ap(t["k"]),
                                scalar1=float(N), scalar2=None,
                                op0=Alu.is_lt).then_inc(sPo, 1)       # ->4
        nc.gpsimd.tensor_scalar(out=ap(t["h1"]), in0=ap(t["k"]),
                                scalar1=0.0, scalar2=None,
                                op0=Alu.is_gt).then_inc(sPo, 1)       # ->5
        nc.gpsimd.wait_ge(sAc, 2)            # z0sum
        nc.gpsimd.tensor_scalar(out=ap(t["s0"]), in0=ap(t["z0sum"]),
                                scalar1=0.0, scalar2=None,
                                op0=Alu.is_gt).then_inc(sPo, 1)       # ->6
        nc.gpsimd.wait_ge(sPo, 6)            # nk + s0 visible
        nc.gpsimd.tensor_mul(ap(t["t2"]), ap(t["nk"]),
                             ap(t["s0"])).then_inc(sPo, 1)            # ->7

        # ---- DVE: a_sel, the z chain, n, rn, maxpool combine -----------
        nc.vector.wait_ge(dA, 16)
        nc.vector.wait_ge(sPo, 2)            # iota
        nc.vector.scalar_tensor_tensor(
            out=ap(t["scr"]), in0=ap(t["iota16"]), scalar=xidx, in1=adj,
            op0=Alu.is_equal, op1=Alu.mult,
            accum_out=t["a_sel"].ap()[:, 0:1]).then_inc(sV, 1)        # ->1
        nc.vector.wait_ge(sV, 1)             # a_sel accum lands async
        nc.vector.tensor_scalar(out=ap(t["asm1"]), in0=ap(t["a_sel"]),
                                scalar1=-1.0, scalar2=None,
                                op0=Alu.add).then_inc(sV, 1)          # ->2
        nc.vector.wait_ge(sV, 2)             # asm1 visible (self)
        nc.vector.wait_ge(sPe, 1)            # mm v half
        nc.vector.tensor_scalar(out=ap(t["t_sb"]),
                                in0=mm.ap()[:, U:2 * U],
                                scalar1=t["asm1"].ap()[:, 0:1],
                                scalar2=None,
                                op0=Alu.mult).then_inc(sV, 1)         # ->3
        nc.vector.wait_ge(sV, 3)             # t_sb visible (self)
        nc.vector.wait_ge(sPe, 2)            # mm u+b half
        nc.vector.tensor_add(ap(t["zz"]), ap(t["t_sb"]),
                             mm.ap()[:, 0:U]).then_inc(sV, 1)         # ->4
        nc.vector.wait_ge(sV, 4)             # zz visible (self)
        nc.vector.wait_ge(sPo, 5)            # h1
        nc.vector.tensor_scalar(out=ap(t["z1"]), in0=ap(t["zz"]),
                                scalar1=t["h1"].ap()[:, 0:1], scalar2=0.0,
                                op0=Alu.mult, op1=Alu.max,
                                accum_out=t["z1sum"].ap()[:, 0:1]
                                ).then_inc(sV, 1)                     # ->5
        nc.vector.wait_ge(sV, 5)             # z1sum accum lands async
        nc.vector.tensor_scalar(out=ap(t["s1"]), in0=ap(t["z1sum"]),
                                scalar1=0.0, scalar2=None,
                                op0=Alu.is_gt).then_inc(sV, 1)        # ->6
        nc.vector.wait_ge(sV, 6)             # s1 visible (self)
        nc.vector.wait_ge(sPo, 7)            # t2
        nc.vector.scalar_tensor_tensor(
            out=ap(t["nn"]), in0=ap(t["k"]),
            scalar=t["s1"].ap()[:, 0:1], in1=ap(t["t2"]),
            op0=Alu.mult, op1=Alu.add).then_inc(sV, 1)                # ->7
        nc.vector.wait_ge(sV, 7)             # nn visible (self)
        nc.vector.reciprocal(ap(t["rn"]), ap(t["nn"])).then_inc(sV, 1)  # ->8
        nc.vector.wait_ge(sAc, 2)            # z0
        nc.vector.wait_ge(sPo, 4)            # h0
        nc.vector.tensor_scalar(out=ap(t["z0h"]), in0=ap(t["z0"]),
                                scalar1=t["h0"].ap()[:, 0:1],
                                scalar2=None,
                                op0=Alu.mult).then_inc(sV, 1)         # ->9 z0h
        nc.vector.wait_ge(sV, 9)             # z0h visible (self)
        nc.vector.tensor_max(t["out_t"].ap()[:, 0:U], ap(t["z1"]),
                             ap(t["z0h"])).then_inc(sV, 1)            # ->10 max

    nc.compile()
    return nc


def get_nc():
    if "nc" not in _CACHE:
        _CACHE["nc"] = _build_nc()
    return _CACHE["nc"]


def make_in_maps(inputs, adj_matrix, xidx, w, b):
    """Shard + pack full inputs into per-core input maps."""
    import ml_dtypes
    f8 = ml_dtypes.float8_e4m3

    x_flat = np.asarray(inputs, dtype=np.float32).reshape(B * N, C)
    adj_flat = np.asarray(adj_matrix, dtype=np.float32).reshape(B * N, N)
    xidx_flat = np.asarray(xidx, dtype=np.int32).reshape(B * N, 1)
    w_full = np.asarray(w, dtype=np.float32)[0]            # [2C, U]
    b_full = np.asarray(b, dtype=np.float32).reshape(1, U)

    wcat = np.concatenate([w_full[0:C, :], w_full[C:2 * C, :]],
                          axis=1).astype(np.float16)       # [C, 2U]
    bb = np.broadcast_to(b_full.astype(np.float16), (P, U))  # [P, U]

    in_maps = []
    for c in range(NCORES):
        rows = slice(c * P, (c + 1) * P)
        x_slab = x_flat[rows]                               # [P, C] f32
        pakA = np.zeros((P, PAKA_W), dtype=np.float16)
        pakA[:, 0:N] = adj_flat[rows]
        pakA[:, N:N + 1] = xidx_flat[rows].astype(np.float16)
        pakB = np.zeros((P, PAKB_W), dtype=np.float16)
        pakB[:, 0:C] = x_slab.T
        pakB[:, C:C + 2 * U] = wcat
        pakB[:, C + 2 * U:C + 2 * U + C] = x_slab
        pakB[:, C + 2 * U + C:C + 2 * U + 2 * C] = bb
        in_maps.append({
            "pakA": np.ascontiguousarray(pakA),
            "pakB": np.ascontiguousarray(pakB),
        })
    return in_maps


def kernel(inputs, adj_matrix, xidx, w, b, _trace=False):
    from concourse.bass_utils import run_bass_kernel_spmd

    nc = get_nc()
    in_maps = make_in_maps(inputs, adj_matrix, xidx, w, b)
    res = run_bass_kernel_spmd(nc, in_maps, list(range(NCORES)),
                               trace=_trace)
    out = np.concatenate([res.results[c]["out"] for c in range(NCORES)],
                         axis=0)
    out = out.astype(np.float32).reshape(B, N, OUTF)
    if _trace:
        _CACHE["last_results"] = res
    return out


# revision 23
# speedup vs baseline: 1.0036x; 1.0036x over previous
"""Trainium2 Bass kernel for nn_EdgeConvolution (gnn_message_passing).

Math
----
Reference (B=2, N=512, C=128, U=128), adj binary {0,1}:
  masked[b,i,j,:]  = adj[b,i,j] * x[b,i,:]
  a_sel[b,i]       = adj[b,i, xidx[b,i]]
  edging[b,i,j,:]  = adj[b,i,j] * [ x_i | (a_sel_i - 1)*x_i ]      (adj^2 = adj)
  out[b,i,j,:]     = relu(adj*(u_i + (a_sel_i-1)*v_i) + b), u = x@W1, v = x@W2
Over j there are only two values per (b,i):
  z1_i = relu(u_i + (a_sel_i-1)*v_i + b)   (edges with adj=1, count k_i)
  z0   = relu(b)                            (edges with adj=0, count N-k_i)
  maxp_i    = max(h1_i*z1_i, h0_i*z0),  h1 = [k>0], h0 = [k<N]
  n_i       = k_i*s1_i + (N-k_i)*s0,  s1 = [any z1>0], s0 = [any z0>0]
  avgpool_i = [ k_i*x_i | k_i*(a_sel_i-1)*x_i ] / n_i
Per-core slab: 128 of the 1024 (b,i) rows; w/b replicated.

Implementation notes
--------------------
- Two packed input DMAs per core: pakA = adj (fp8 e4m3, exact for 0/1) on
  the SP HWDGE ring; pakB = [x^T | Wcat | x | b-broadcast | xidx] in fp16
  on the ACT ring.
- +b is folded into the matmul as a rank-1 accumulate (ones x b_row).
- h1 is folded into the relu (z1 = max(zz*h1, 0)); h0 scales z0 on ACT.
- Single full-row output store issued from ACT; no engine waits on the
  store completion — its receipt latency hides under the runtime's fixed
  end-of-NEFF semaphore-zeroing postamble.
- No same-engine self-waits: engine execution is serial in-order (accum
  reads materialize via in-stream READ_ACCUMULATOR), so only cross-engine
  and DMA waits are needed.
"""

import numpy as np

B, N, C, U = 2, 512, 128, 128
P = 128          # rows (b,i) per core == SBUF partitions
NCORES = 8
OUTF = U + 2 * C  # 384
PAKA_W = 528     # adj 512 | xidx 1 | pad 15   (fp16)
PAKB_W = 640     # xT 128 | wcat 256 | x 128 | b_bcast 128

_CACHE: dict = {}


def _build_nc():
    import concourse.bacc as bacc
    import concourse.mybir as mybir

    f32 = mybir.dt.float32
    f16 = mybir.dt.float16
    f8 = mybir.dt.float8e4
    Alu = mybir.AluOpType
    Act = mybir.ActivationFunctionType

    nc = bacc.Bacc("TRN2", target_bir_lowering=False, debug=False,
                   num_devices=NCORES)

    pakA_d = nc.dram_tensor("pakA", [P, PAKA_W], f16, kind="ExternalInput")
    pakB_d = nc.dram_tensor("pakB", [P, PAKB_W], f16, kind="ExternalInput")
    out_d = nc.dram_tensor("out", [P, OUTF], f16, kind="ExternalOutput")

    ctx_tensors = [
        ("pakA_t", [P, PAKA_W], f16), ("pakB_t", [P, PAKB_W], f16),
        ("iota16", [P, N], f16), ("ones16", [1, P], f16),
        ("scr", [P, N], f16), ("scr2", [P, N], f16), ("warm", [P, 1], f32),
        ("a_sel", [P, 1], f32), ("asm1", [P, 1], f32), ("k", [P, 1], f32),
        ("t_sb", [P, U], f32),
        ("zz", [P, U], f32), ("z1", [P, U], f16), ("z1sum", [P, 1], f32),
        ("z0", [P, U], f32), ("z0sum", [P, 1], f32), ("z0h", [P, U], f16),
        ("s0", [P, 1], f32), ("s1", [P, 1], f32), ("nk", [P, 1], f32),
        ("h0", [P, 1], f32), ("h1", [P, 1], f32), ("t2", [P, 1], f32),
        ("nn", [P, 1], f32), ("rn", [P, 1], f32),
        ("xcat", [P, 2 * C], f16),
        ("out_t", [P, OUTF], f16),
    ]

    from contextlib import ExitStack
    with ExitStack() as ctx:
        t = {}
        for name, shape, dt in ctx_tensors:
            t[name] = ctx.enter_context(nc.sbuf_tensor(name, shape, dt))
        mm = ctx.enter_context(nc.psum_tensor("mm", [P, 2 * U], f32))

        dA = ctx.enter_context(nc.semaphore("dA"))
        dB = ctx.enter_context(nc.semaphore("dB"))
        dS = ctx.enter_context(nc.semaphore("dS"))    # stores; never waited
        sV = ctx.enter_context(nc.semaphore("sV"))
        sPo = ctx.enter_context(nc.semaphore("sPo"))
        sAc = ctx.enter_context(nc.semaphore("sAc"))
        sPe = ctx.enter_context(nc.semaphore("sPe"))

        ap = lambda h: h.ap()
        adj = t["pakA_t"].ap()[:, 0:N]
        xT = t["pakB_t"].ap()[:, 0:C]
        wcat = t["pakB_t"].ap()[:, C:C + 2 * U]
        xrow = t["pakB_t"].ap()[:, C + 2 * U:C + 2 * U + C]
        bb = t["pakB_t"].ap()[:, C + 2 * U + C:C + 2 * U + 2 * C]
        brow = t["pakB_t"].ap()[0:1, C + 2 * U + C:C + 2 * U + 2 * C]
        xidx = t["pakA_t"].ap()[:, N:N + 1]
        c0 = nc.const_aps.aps[(f32, 0.0)]

        # ---- SP: input DMA A only --------------------------------------
        nc.sync.dma_start(ap(t["pakA_t"]), pakA_d.ap()).then_inc(dA, 16)

        # ---- ACT: input DMA B, k, z0 path, xcat scales, avg, store -----
        nc.scalar.dma_start(ap(t["pakB_t"]), pakB_d.ap()).then_inc(dB, 16)
        # warm the activation table during the DMA wait
        nc.scalar.activation(out=ap(t["warm"]), in_=c0, func=Act.Relu,
                             bias=c0[:, 0:1])
        nc.scalar.wait_ge(dA, 16)
        nc.scalar.activation(out=ap(t["scr2"]), in_=adj, func=Act.Copy,
                             accum_out=t["k"].ap()[:, 0:1])           # k
        nc.scalar.wait_ge(dB, 16)
        nc.scalar.activation(out=ap(t["z0"]), in_=bb, func=Act.Relu,
                             bias=c0[:, 0:1],
                             accum_out=t["z0sum"].ap()[:, 0:1]
                             ).then_inc(sAc, 1)          # ->1 z0 + k-accum-done
        nc.scalar.activation(out=t["xcat"].ap()[:, 0:C], in_=xrow,
                             func=Act.Copy, scale=t["k"].ap()[:, 0:1]
                             ).then_inc(sAc, 1)          # ->2 xk + z0sum-done
        nc.scalar.wait_ge(sAc, 2)            # xk visible (self)
        nc.scalar.wait_ge(sV, 2)             # asm1
        nc.scalar.activation(out=t["xcat"].ap()[:, C:2 * C],
                             in_=t["xcat"].ap()[:, 0:C],
                             func=Act.Copy, scale=t["asm1"].ap()[:, 0:1]
                             ).then_inc(sAc, 1)                       # ->3 xcat2
        nc.scalar.wait_ge(sAc, 3)            # xcat2 visible (self)
        nc.scalar.wait_ge(sV, 8)             # rn
        nc.scalar.activation(out=t["out_t"].ap()[:, U:OUTF],
                             in_=ap(t["xcat"]),
                             func=Act.Copy, scale=t["rn"].ap()[:, 0:1]
                             ).then_inc(sAc, 1)                       # ->4 avg
        nc.scalar.wait_ge(sAc, 4)            # avg visible (self)
        nc.scalar.wait_ge(sV, 10)            # out_max written by DVE
        nc.scalar.dma_start(out_d.ap(), ap(t["out_t"])).then_inc(dS, 16)

        # ---- PE: mm = [x@W1 + b | x@W2], v half first -------------------
        nc.tensor.wait_ge(dB, 16)
        nc.tensor.matmul(mm.ap()[:, U:2 * U], lhsT=xT,
                         rhs=wcat[:, U:2 * U],
                         start=True, stop=True).then_inc(sPe, 1)      # ->1 v
        nc.tensor.matmul(mm.ap()[:, 0:U], lhsT=xT, rhs=wcat[:, 0:U],
                         start=True, stop=False)
        nc.tensor.wait_ge(sPo, 1)            # ones16
        nc.tensor.matmul(mm.ap()[:, 0:U], lhsT=ap(t["ones16"]), rhs=brow,
                         start=False, stop=True,
                         skip_group_check=True).then_inc(sPe, 1)      # ->2 u+b

        # ---- GPSIMD: iota + k-derived scalars --------------------------
        nc.gpsimd.memset(ap(t["ones16"]), 1.0).then_inc(sPo, 1)       # ->1
        nc.gpsimd.iota(ap(t["iota16"]), pattern=[[1, N]], base=0,
                       channel_multiplier=0,
                       allow_small_or_imprecise_dtypes=True
                       ).then_inc(sPo, 1)                             # ->2
        nc.gpsimd.wait_ge(sAc, 1)            # k accum materialized
        nc.gpsimd.tensor_scalar(out=ap(t["nk"]), in0=ap(t["k"]),
                                scalar1=-1.0, scalar2=float(N),
                                op0=Alu.mult, op1=Alu.add
                                ).then_inc(sPo, 1)                    # ->3
        nc.gpsimd.tensor_scalar(out=ap(t["h0"]), in0=ap(t["k"]),
                                scalar1=float(N), scalar2=None,
                                op0=Alu.is_lt).then_inc(sPo, 1)       # ->4
        nc.gpsimd.tensor_scalar(out=ap(t["h1"]), in0=ap(t["k"]),
                                scalar1=0.0, scalar2=None,
                                op0=Alu.is_gt).then_inc(sPo, 1)       # ->5
        nc.gpsimd.wait_ge(sAc, 2)            # z0sum materialized
        nc.gpsimd.tensor_scalar(out=ap(t["s0"]), in0=ap(t["z0sum"]),
                                scalar1=0.0, scalar2=None,
                                op0=Alu.is_gt).then_inc(sPo, 1)       # ->6
        nc.gpsimd.wait_ge(sPo, 6)            # nk + s0 visible
        nc.gpsimd.tensor_mul(ap(t["t2"]), ap(t["nk"]),
                             ap(t["s0"])).then_inc(sPo, 1)            # ->7

        # ---- DVE: a_sel, the z chain, n, rn, maxpool combine -----------
        nc.vector.wait_ge(dA, 16)
        nc.vector.wait_ge(sPo, 2)            # iota
        nc.vector.scalar_tensor_tensor(
            out=ap(t["scr"]), in0=ap(t["iota16"]), scalar=xidx, in1=adj,
            op0=Alu.is_equal, op1=Alu.mult,
            accum_out=t["a_sel"].ap()[:, 0:1]).then_inc(sV, 1)        # ->1
        nc.vector.wait_ge(sV, 1)             # a_sel accum lands async
        nc.vector.tensor_scalar(out=ap(t["asm1"]), in0=ap(t["a_sel"]),
                                scalar1=-1.0, scalar2=None,
                                op0=Alu.add).then_inc(sV, 1)          # ->2
        nc.vector.wait_ge(sV, 2)             # asm1 visible (self)
        nc.vector.wait_ge(sPe, 1)            # mm v half
        nc.vector.tensor_scalar(out=ap(t["t_sb"]),
                                in0=mm.ap()[:, U:2 * U],
                                scalar1=t["asm1"].ap()[:, 0:1],
                                scalar2=None,
                                op0=Alu.mult).then_inc(sV, 1)         # ->3
        nc.vector.wait_ge(sV, 3)             # t_sb visible (self)
        nc.vector.wait_ge(sPe, 2)            # mm u+b half
        nc.vector.tensor_add(ap(t["zz"]), ap(t["t_sb"]),
                             mm.ap()[:, 0:U]).then_inc(sV, 1)         # ->4
        nc.vector.wait_ge(sV, 4)             # zz visible (self)
        nc.vector.wait_ge(sPo, 5)            # h1
        nc.vector.tensor_scalar(out=ap(t["z1"]), in0=ap(t["zz"]),
                                scalar1=t["h1"].ap()[:, 0:1], scalar2=0.0,
                                op0=Alu.mult, op1=Alu.max,
                                accum_out=t["z1sum"].ap()[:, 0:1]
                                ).then_inc(sV, 1)                     # ->5
        nc.vector.wait_ge(sV, 5)             # z1sum accum lands async
        nc.vector.tensor_scalar(out=ap(t["s1"]), in0=ap(t["z1sum"]),
                                scalar1=0.0, scalar2=None,
                                op0=Alu.is_gt).then_inc(sV, 1)        # ->6
        nc.vector.wait_ge(sV, 6)             # s1 visible (self)
        nc.vector.wait_ge(sPo, 7)            # t2
        nc.vector.scalar_tensor_tensor(
            out=ap(t["nn"]), in0=ap(t["k"]),
            scalar=t["s1"].ap()[:, 0:1], in1=ap(t["t2"]),
            op0=Alu.mult, op1=Alu.add).then_inc(sV, 1)                # ->7
        nc.vector.wait_ge(sV, 7)             # nn visible (self)
        nc.vector.reciprocal(ap(t["rn"]), ap(t["nn"])).then_inc(sV, 1)  # ->8
        nc.vector.wait_ge(sAc, 1)            # z0
        nc.vector.wait_ge(sPo, 4)            # h0
        nc.vector.tensor_scalar(out=ap(t["z0h"]), in0=ap(t["z0"]),
                                scalar1=t["h0"].ap()[:, 0:1],
                                scalar2=None,
                                op0=Alu.mult).then_inc(sV, 1)         # ->9 z0h
        nc.vector.wait_ge(sV, 9)             # z0h visible (self)
        nc.vector.tensor_max(t["out_t"].ap()[:, 0:U], ap(t["z1"]),
                             ap(t["z0h"])).then_inc(sV, 1)            # ->10 max

    nc.compile()
    return nc


def get_nc():
    if "nc" not in _CACHE:
        _CACHE["nc"] = _build_nc()
    return _CACHE["nc"]


def make_in_maps(inputs, adj_matrix, xidx, w, b):
    """Shard + pack full inputs into per-core input maps."""
    import ml_dtypes
    f8 = ml_dtypes.float8_e4m3

    x_flat = np.asarray(inputs, dtype=np.float32).reshape(B * N, C)
    adj_flat = np.asarray(adj_matrix, dtype=np.float32).reshape(B * N, N)
    xidx_flat = np.asarray(xidx, dtype=np.int32).reshape(B * N, 1)
    w_full = np.asarray(w, dtype=np.float32)[0]            # [2C, U]
    b_full = np.asarray(b, dtype=np.float32).reshape(1, U)

    wcat = np.concatenate([w_full[0:C, :], w_full[C:2 * C, :]],
                          axis=1).astype(np.float16)       # [C, 2U]
    bb = np.broadcast_to(b_full.astype(np.float16), (P, U))  # [P, U]

    in_maps = []
    for c in range(NCORES):
        rows = slice(c * P, (c + 1) * P)
        x_slab = x_flat[rows]                               # [P, C] f32
        pakA = np.zeros((P, PAKA_W), dtype=np.float16)
        pakA[:, 0:N] = adj_flat[rows]
        pakA[:, N:N + 1] = xidx_flat[rows].astype(np.float16)
        pakB = np.zeros((P, PAKB_W), dtype=np.float16)
        pakB[:, 0:C] = x_slab.T
        pakB[:, C:C + 2 * U] = wcat
        pakB[:, C + 2 * U:C + 2 * U + C] = x_slab
        pakB[:, C + 2 * U + C:C + 2 * U + 2 * C] = bb
        in_maps.append({
            "pakA": np.ascontiguousarray(pakA),
            "pakB": np.ascontiguousarray(pakB),
        })
    return in_maps


def kernel(inputs, adj_matrix, xidx, w, b, _trace=False):
    from concourse.bass_utils import run_bass_kernel_spmd

    nc = get_nc()
    in_maps = make_in_maps(inputs, adj_matrix, xidx, w, b)
    res = run_bass_kernel_spmd(nc, in_maps, list(range(NCORES)),
                               trace=_trace)
    out = np.concatenate([res.results[c]["out"] for c in range(NCORES)],
                         axis=0)
    out = out.astype(np.float32).reshape(B, N, OUTF)
    if _trace:
        _CACHE["last_results"] = res
    return out


# revision 26
# speedup vs baseline: 1.0154x; 1.0118x over previous
"""Trainium2 Bass kernel for nn_EdgeConvolution (gnn_message_passing).

Math
----
Reference (B=2, N=512, C=128, U=128), adj binary {0,1}:
  masked[b,i,j,:]  = adj[b,i,j] * x[b,i,:]
  a_sel[b,i]       = adj[b,i, xidx[b,i]]
  edging[b,i,j,:]  = adj[b,i,j] * [ x_i | (a_sel_i - 1)*x_i ]      (adj^2 = adj)
  out[b,i,j,:]     = relu(adj*(u_i + (a_sel_i-1)*v_i) + b), u = x@W1, v = x@W2
Over j there are only two values per (b,i):
  z1_i = relu(u_i + (a_sel_i-1)*v_i + b)   (edges with adj=1, count k_i)
  z0   = relu(b)                            (edges with adj=0, count N-k_i)
  maxp_i    = max(h1_i*z1_i, h0_i*z0),  h1 = [k>0], h0 = [k<N]
  n_i       = k_i*s1_i + (N-k_i)*s0,  s1 = [any z1>0], s0 = [any z0>0]
  avgpool_i = [ k_i*x_i | k_i*(a_sel_i-1)*x_i ] / n_i
Per-core slab: 128 of the 1024 (b,i) rows; w/b replicated.

Implementation notes
--------------------
- Two packed input DMAs per core: pakA = adj (fp8 e4m3, exact for 0/1) on
  the SP HWDGE ring; pakB = [x^T | Wcat | x | b-broadcast | xidx] in fp16
  on the ACT ring.
- +b is folded into the matmul as a rank-1 accumulate (ones x b_row).
- h1 is folded into the relu (z1 = max(zz*h1, 0)); h0 scales z0 on ACT.
- Single full-row output store issued from ACT; no engine waits on the
  store completion — its receipt latency hides under the runtime's fixed
  end-of-NEFF semaphore-zeroing postamble.
- No same-engine self-waits: engine execution is serial in-order (accum
  reads materialize via in-stream READ_ACCUMULATOR), so only cross-engine
  and DMA waits are needed.
"""

import numpy as np

B, N, C, U = 2, 512, 128, 128
P = 128          # rows (b,i) per core == SBUF partitions
NCORES = 8
OUTF = U + 2 * C  # 384
PAKA_W = 528     # adj 512 | xidx 1 | pad 15   (fp16)
PAKB_W = 640     # xT 128 | wcat 256 | x 128 | b_bcast 128

_CACHE: dict = {}


def _build_nc():
    import concourse.bacc as bacc
    import concourse.mybir as mybir

    f32 = mybir.dt.float32
    f16 = mybir.dt.float16
    f8 = mybir.dt.float8e4
    Alu = mybir.AluOpType
    Act = mybir.ActivationFunctionType

    nc = bacc.Bacc("TRN2", target_bir_lowering=False, debug=False,
                   num_devices=NCORES)

    pakA_d = nc.dram_tensor("pakA", [P, PAKA_W], f16, kind="ExternalInput")
    pakB_d = nc.dram_tensor("pakB", [P, PAKB_W], f16, kind="ExternalInput")
    out_d = nc.dram_tensor("out", [P, OUTF], f16, kind="ExternalOutput")

    ctx_tensors = [
        ("pakA_t", [P, PAKA_W], f16), ("pakB_t", [P, PAKB_W], f16),
        ("iota16", [P, N], f16), ("ones16", [1, P], f16),
        ("scr", [P, N], f16), ("scr2", [P, N], f16), ("warm", [P, 1], f32),
        ("a_sel", [P, 1], f32), ("asm1", [P, 1], f32), ("k", [P, 1], f32),
        ("t_sb", [P, U], f32),
        ("zz", [P, U], f32), ("z1", [P, U], f16), ("z1sum", [P, 1], f32),
        ("z0", [P, U], f32), ("z0sum", [P, 1], f32), ("z0h", [P, U], f16),
        ("s0", [P, 1], f32), ("s1", [P, 1], f32), ("nk", [P, 1], f32),
        ("h0", [P, 1], f32), ("h1", [P, 1], f32), ("t2", [P, 1], f32),
        ("nn", [P, 1], f32), ("rn", [P, 1], f32),
        ("xcat", [P, 2 * C], f16),
        ("out_t", [P, OUTF], f16),
    ]

    from contextlib import ExitStack
    with ExitStack() as ctx:
        t = {}
        for name, shape, dt in ctx_tensors:
            t[name] = ctx.enter_context(nc.sbuf_tensor(name, shape, dt))
        mm = ctx.enter_context(nc.psum_tensor("mm", [P, 2 * U], f32))

        dA = ctx.enter_context(nc.semaphore("dA"))
        dB = ctx.enter_context(nc.semaphore("dB"))
        dS = ctx.enter_context(nc.semaphore("dS"))    # stores; never waited
        sV = ctx.enter_context(nc.semaphore("sV"))
        sPo = ctx.enter_context(nc.semaphore("sPo"))
        sAc = ctx.enter_context(nc.semaphore("sAc"))
        sPe = ctx.enter_context(nc.semaphore("sPe"))

        ap = lambda h: h.ap()
        adj = t["pakA_t"].ap()[:, 0:N]
        xT = t["pakB_t"].ap()[:, 0:C]
        wcat = t["pakB_t"].ap()[:, C:C + 2 * U]
        xrow = t["pakB_t"].ap()[:, C + 2 * U:C + 2 * U + C]
        bb = t["pakB_t"].ap()[:, C + 2 * U + C:C + 2 * U + 2 * C]
        brow = t["pakB_t"].ap()[0:1, C + 2 * U + C:C + 2 * U + 2 * C]
        xidx = t["pakA_t"].ap()[:, N:N + 1]
        c0 = nc.const_aps.aps[(f32, 0.0)]

        # ---- SP: input DMA A only --------------------------------------
        nc.sync.dma_start(ap(t["pakA_t"]), pakA_d.ap()).then_inc(dA, 16)

        # ---- ACT: input DMA B, k, z0 path, xcat scales, avg, store -----
        nc.scalar.dma_start(ap(t["pakB_t"]), pakB_d.ap()).then_inc(dB, 16)
        # warm the activation table during the DMA wait
        nc.scalar.activation(out=ap(t["warm"]), in_=c0, func=Act.Relu,
                             bias=c0[:, 0:1])
        nc.scalar.wait_ge(dA, 16)
        nc.scalar.activation(out=ap(t["scr2"]), in_=adj, func=Act.Copy,
                             accum_out=t["k"].ap()[:, 0:1])           # k
        nc.scalar.wait_ge(dB, 16)
        nc.scalar.activation(out=ap(t["z0"]), in_=bb, func=Act.Relu,
                             bias=c0[:, 0:1],
                             accum_out=t["z0sum"].ap()[:, 0:1]
                             ).then_inc(sAc, 1)          # ->1 z0 + k-accum-done
        nc.scalar.activation(out=t["xcat"].ap()[:, 0:C], in_=xrow,
                             func=Act.Copy, scale=t["k"].ap()[:, 0:1]
                             ).then_inc(sAc, 1)          # ->2 xk + z0sum-done
        nc.scalar.wait_ge(sAc, 2)            # xk visible (self)
        nc.scalar.wait_ge(sV, 2)             # asm1
        nc.scalar.activation(out=t["xcat"].ap()[:, C:2 * C],
                             in_=t["xcat"].ap()[:, 0:C],
                             func=Act.Copy, scale=t["asm1"].ap()[:, 0:1]
                             ).then_inc(sAc, 1)                       # ->3 xcat2
        nc.scalar.wait_ge(sAc, 3)            # xcat2 visible (self)
        nc.scalar.wait_ge(sV, 8)             # rn
        nc.scalar.activation(out=t["out_t"].ap()[:, U:OUTF],
                             in_=ap(t["xcat"]),
                             func=Act.Copy, scale=t["rn"].ap()[:, 0:1]
                             ).then_inc(sAc, 1)                       # ->4 avg
        nc.scalar.wait_ge(sAc, 4)            # avg visible (self)
        nc.scalar.wait_ge(sV, 10)            # out_max written by DVE
        nc.scalar.dma_start(out_d.ap(), ap(t["out_t"])).then_inc(dS, 16)

        # ---- PE: mm = [x@W1 + b | x@W2], v half first -------------------
        nc.tensor.wait_ge(dB, 16)
        nc.tensor.matmul(mm.ap()[:, U:2 * U], lhsT=xT,
                         rhs=wcat[:, U:2 * U],
                         start=True, stop=True).then_inc(sPe, 1)      # ->1 v
        nc.tensor.matmul(mm.ap()[:, 0:U], lhsT=xT, rhs=wcat[:, 0:U],
                         start=True, stop=False)
        nc.tensor.wait_ge(sPo, 2)            # ones16
        nc.tensor.matmul(mm.ap()[:, 0:U], lhsT=ap(t["ones16"]), rhs=brow,
                         start=False, stop=True,
                         skip_group_check=True).then_inc(sPe, 1)      # ->2 u+b

        # ---- GPSIMD: iota + k-derived scalars.  Writes are only globally
        # visible after a drain (8 async Q7 cores), so cross-engine signals
        # come from drain instructions.
        nc.gpsimd.memset(ap(t["ones16"]), 1.0)
        nc.gpsimd.iota(ap(t["iota16"]), pattern=[[1, N]], base=0,
                       channel_multiplier=0,
                       allow_small_or_imprecise_dtypes=True)
        nc.gpsimd.drain().then_inc(sPo, 2)                            # ->2
        nc.gpsimd.wait_ge(sAc, 1)            # k accum materialized
        nc.gpsimd.tensor_scalar(out=ap(t["nk"]), in0=ap(t["k"]),
                                scalar1=-1.0, scalar2=float(N),
                                op0=Alu.mult, op1=Alu.add)
        nc.gpsimd.tensor_scalar(out=ap(t["h0"]), in0=ap(t["k"]),
                                scalar1=float(N), scalar2=None,
                                op0=Alu.is_lt)
        nc.gpsimd.tensor_scalar(out=ap(t["h1"]), in0=ap(t["k"]),
                                scalar1=0.0, scalar2=None,
                                op0=Alu.is_gt)
        nc.gpsimd.drain().then_inc(sPo, 3)                            # ->5
        nc.gpsimd.wait_ge(sAc, 2)            # z0sum materialized
        nc.gpsimd.tensor_scalar(out=ap(t["s0"]), in0=ap(t["z0sum"]),
                                scalar1=0.0, scalar2=None,
                                op0=Alu.is_gt)
        nc.gpsimd.tensor_mul(ap(t["t2"]), ap(t["nk"]), ap(t["s0"]))
        nc.gpsimd.drain().then_inc(sPo, 2)                            # ->7

        # ---- DVE: a_sel, the z chain, n, rn, maxpool combine -----------
        nc.vector.wait_ge(dA, 16)
        nc.vector.wait_ge(sPo, 2)            # iota
        nc.vector.scalar_tensor_tensor(
            out=ap(t["scr"]), in0=ap(t["iota16"]), scalar=xidx, in1=adj,
            op0=Alu.is_equal, op1=Alu.mult,
            accum_out=t["a_sel"].ap()[:, 0:1]).then_inc(sV, 1)        # ->1
        nc.vector.wait_ge(sV, 1)             # a_sel accum lands async
        nc.vector.tensor_scalar(out=ap(t["asm1"]), in0=ap(t["a_sel"]),
                                scalar1=-1.0, scalar2=None,
                                op0=Alu.add).then_inc(sV, 1)          # ->2
        nc.vector.wait_ge(sV, 2)             # asm1 visible (self)
        nc.vector.wait_ge(sPe, 1)            # mm v half
        nc.vector.tensor_scalar(out=ap(t["t_sb"]),
                                in0=mm.ap()[:, U:2 * U],
                                scalar1=t["asm1"].ap()[:, 0:1],
                                scalar2=None,
                                op0=Alu.mult).then_inc(sV, 1)         # ->3
        nc.vector.wait_ge(sV, 3)             # t_sb visible (self)
        nc.vector.wait_ge(sPe, 2)            # mm u+b half
        nc.vector.tensor_add(ap(t["zz"]), ap(t["t_sb"]),
                             mm.ap()[:, 0:U]).then_inc(sV, 1)         # ->4
        nc.vector.wait_ge(sV, 4)             # zz visible (self)
        nc.vector.wait_ge(sPo, 5)            # h1
        nc.vector.tensor_scalar(out=ap(t["z1"]), in0=ap(t["zz"]),
                                scalar1=t["h1"].ap()[:, 0:1], scalar2=0.0,
                                op0=Alu.mult, op1=Alu.max,
                                accum_out=t["z1sum"].ap()[:, 0:1]
                                ).then_inc(sV, 1)                     # ->5
        nc.vector.wait_ge(sV, 5)             # z1sum accum lands async
        nc.vector.tensor_scalar(out=ap(t["s1"]), in0=ap(t["z1sum"]),
                                scalar1=0.0, scalar2=None,
                                op0=Alu.is_gt).then_inc(sV, 1)        # ->6
        nc.vector.wait_ge(sV, 6)             # s1 visible (self)
        nc.vector.wait_ge(sPo, 7)            # t2
        nc.vector.scalar_tensor_tensor(
            out=ap(t["nn"]), in0=ap(t["k"]),
            scalar=t["s1"].ap()[:, 0:1], in1=ap(t["t2"]),
            op0=Alu.mult, op1=Alu.add).then_inc(sV, 1)                # ->7
        nc.vector.wait_ge(sV, 7)             # nn visible (self)
        nc.vector.reciprocal(ap(t["rn"]), ap(t["nn"])).then_inc(sV, 1)  # ->8
        nc.vector.wait_ge(sAc, 1)            # z0
        nc.vector.wait_ge(sPo, 5)            # h0 (post-drain)
        nc.vector.tensor_scalar(out=ap(t["z0h"]), in0=ap(t["z0"]),
                                scalar1=t["h0"].ap()[:, 0:1],
                                scalar2=None,
                                op0=Alu.mult).then_inc(sV, 1)         # ->9 z0h
        nc.vector.wait_ge(sV, 9)             # z0h visible (self)
        nc.vector.tensor_max(t["out_t"].ap()[:, 0:U], ap(t["z1"]),
                             ap(t["z0h"])).then_inc(sV, 1)            # ->10 max

    nc.compile()
    return nc


def get_nc():
    if "nc" not in _CACHE:
        _CACHE["nc"] = _build_nc()
    return _CACHE["nc"]


def make_in_maps(inputs, adj_matrix, xidx, w, b):
    """Shard + pack full inputs into per-core input maps."""
    import ml_dtypes
    f8 = ml_dtypes.float8_e4m3

    x_flat = np.asarray(inputs, dtype=np.float32).reshape(B * N, C)
    adj_flat = np.asarray(adj_matrix, dtype=np.float32).reshape(B * N, N)
    xidx_flat = np.asarray(xidx, dtype=np.int32).reshape(B * N, 1)
    w_full = np.asarray(w, dtype=np.float32)[0]            # [2C, U]
    b_full = np.asarray(b, dtype=np.float32).reshape(1, U)

    wcat = np.concatenate([w_full[0:C, :], w_full[C:2 * C, :]],
                          axis=1).astype(np.float16)       # [C, 2U]
    bb = np.broadcast_to(b_full.astype(np.float16), (P, U))  # [P, U]

    in_maps = []
    for c in range(NCORES):
        rows = slice(c * P, (c + 1) * P)
        x_slab = x_flat[rows]                               # [P, C] f32
        pakA = np.zeros((P, PAKA_W), dtype=np.float16)
        pakA[:, 0:N] = adj_flat[rows]
        pakA[:, N:N + 1] = xidx_flat[rows].astype(np.float16)
        pakB = np.zeros((P, PAKB_W), dtype=np.float16)
        pakB[:, 0:C] = x_slab.T
        pakB[:, C:C + 2 * U] = wcat
        pakB[:, C + 2 * U:C + 2 * U + C] = x_slab
        pakB[:, C + 2 * U + C:C + 2 * U + 2 * C] = bb
        in_maps.append({
            "pakA": np.ascontiguousarray(pakA),
            "pakB": np.ascontiguousarray(pakB),
        })
    return in_maps


def kernel(inputs, adj_matrix, xidx, w, b, _trace=False):
    from concourse.bass_utils import run_bass_kernel_spmd

    nc = get_nc()
    in_maps = make_in_maps(inputs, adj_matrix, xidx, w, b)
    res = run_bass_kernel_spmd(nc, in_maps, list(range(NCORES)),
                               trace=_trace)
    out = np.concatenate([res.results[c]["out"] for c in range(NCORES)],
                         axis=0)
    out = out.astype(np.float32).reshape(B, N, OUTF)
    if _trace:
        _CACHE["last_results"] = res
    return out
